# revision 22
# baseline (speedup 1.0000x reference)
"""LIF neuron scan kernel for Trainium2 (8 NeuronCores, raw Bass SPMD).

Math (per timestep): v = v_prev*0.5 + x + r; s = (v > 0); v *= (1-s).
Reset+leak fold to v = 0.5*min(v_prev, 0) + (x + r).  With block-local
power-of-two rescaling (block size K=10): within block, state w_i = 2^i*v
obeys the mult-free step  w_i = min(w_{i-1}, 0) + u_i  with
u_i = 2^i*(x+r) (prescaled on host; exact fp16 exponent shifts, values
bounded by ~2^10*20 << fp16 max).  At block boundaries the carried state
rescales by 2^-K:  w'_0 = min(w_9*2^-K + u_0, u_0)  (two fused DVE ops).
Inputs ship as fp16 (validated: rel err ~9e-3 vs the 2e-2 gate), halving
HBM traffic; spikes s = Sign(w) > 0 are unaffected by the scaling.

The serial time recurrence runs step-wise on the DVE as TWO interleaved
independent chains (feature halves), so adjacent instructions have no
data dependency and the engine pipeline stays full.  GpSimd is kept
completely idle (its big ops starve concurrent DVE work; measured).

Sharding: data-parallel along batch; core i gets inp[:, 8i:8i+8, :].
Per-core layout [128 partitions, T, F=128 features] fp16, time-major.
Input DMAs alternate between both hardware DGE queues (SP and Act);
spikes (uint8 via Act's saturating Sign cast) DMA out on the Act queue.

Write-visibility discipline (observed on HW): an engine's posted SBUF
writes can lag its semaphore increment, so cross-engine consumers wait
with a one-producer-chunk lag (sign(c) waits dve_done >= c+2, final
chunk covered by DVE's trailing drain-inc; the out-DMA for chunk c is
issued after sign(c+1), tail after act.drain()).
"""
import sys
sys.path.insert(0, "/opt/trn_rl_repo")
import numpy as np
import concourse.bass as bass
from concourse import mybir
from concourse.bass_utils import run_bass_kernel_spmd

F16 = mybir.dt.float16
U8 = mybir.dt.uint8
T, B, N = 100, 64, 2048
NCORES = 8
B_LOC = B // NCORES
P = 128
F = (B_LOC * N) // P          # 128 features per partition
K = 10                        # rescale block size
CHUNK_STEPS = (2, 4, 8, 10, 12, 12, 12, 12, 14, 14)   # DMA chunks, ramped
NCHUNK = len(CHUNK_STEPS)
CH_OFF = [sum(CHUNK_STEPS[:i]) for i in range(NCHUNK + 1)]
H = F // 2                    # feature half per chain
RESC = float(2.0 ** -K)
SP_CHUNKS = (0, 2, 4, 6, 8)
ACT_CHUNKS = (1, 3, 5, 7, 9)
# dve_done increments: chunk-granular until t=79, then every 5 steps so the
# tail signs can chase the last steps closely.  12 incs + trailing drain-inc.
INC_STEPS = (9, 19, 29, 39, 49, 59, 69, 79, 84, 89, 94, 99)
# sign/out pieces: (start_t, n_steps, dve_done wait value = data inc + 1 lag)
PIECES = [(c * K, K, min(c + 2, 13)) for c in range(8)] + \
         [(80, 5, 10), (85, 5, 11), (90, 5, 12), (95, 5, 13)]


def _build_nc():
    nc = bass.Bass()
    u_ext = nc.dram_tensor("u", [P, T * F], F16, kind="ExternalInput")
    s_ext = nc.dram_tensor("s", [P, T * F], U8, kind="ExternalOutput")

    with (
        nc.sbuf_tensor([P, T * F], F16) as ub,
        nc.sbuf_tensor([P, T * F], F16) as wb,
        nc.sbuf_tensor([P, T * F], U8) as sb,
        nc.sbuf_tensor([P, F], F16, side="right") as z0,
        nc.sbuf_tensor([P, F], F16, side="right") as tmp,
        nc.semaphore() as sem_sp,
        nc.semaphore() as sem_act,
        nc.semaphore() as dve_done,
        nc.semaphore() as sem_out,
        nc.Block(no_gpsimd_drain=True) as block,
    ):
        def in_dep(c):
            if c in SP_CHUNKS:
                return sem_sp, 16 * (SP_CHUNKS.index(c) + 1)
            return sem_act, 16 * (ACT_CHUNKS.index(c) + 1)

        @block.sync
        def _(sync):
            for c in SP_CHUNKS:
                lo, sz = CH_OFF[c] * F, CHUNK_STEPS[c] * F
                sync.dma_start(ub[:, lo:lo+sz], u_ext[:, lo:lo+sz]).then_inc(sem_sp, 16)

        @block.vector
        def _(vector):
            vector.memset(z0[:], 0.0)
            for c in range(NCHUNK):
                sem, cnt = in_dep(c)
                vector.wait_ge(sem, cnt)
                for t in range(CH_OFF[c], CH_OFF[c + 1]):
                    if t % K == 0 and t > 0:
                        # boundary: w = min(wprev*2^-K + u, u); emit both
                        # chains' stt halves first so the dependent mins
                        # are not back-to-back with their producers
                        for h in range(2):
                            sl = slice(t*F + h*H, t*F + h*H + H)
                            wprev = wb[:, (t-1)*F + h*H:(t-1)*F + h*H + H]
                            nc.vector.scalar_tensor_tensor(
                                tmp[:, h*H:(h+1)*H], wprev, RESC, ub[:, sl],
                                mybir.AluOpType.mult, mybir.AluOpType.add)
                        for h in range(2):
                            sl = slice(t*F + h*H, t*F + h*H + H)
                            nc.vector.tensor_tensor(
                                wb[:, sl], tmp[:, h*H:(h+1)*H], ub[:, sl],
                                mybir.AluOpType.min)
                        continue
                    for h in range(2):
                        sl = slice(t*F + h*H, t*F + h*H + H)
                        if t == 0:
                            wprev = z0[:, h*H:(h+1)*H]
                        else:
                            wprev = wb[:, (t-1)*F + h*H:(t-1)*F + h*H + H]
                        ins = nc.vector.scalar_tensor_tensor(
                            wb[:, sl], wprev, 0.0, ub[:, sl],
                            mybir.AluOpType.min, mybir.AluOpType.add)
                        if t in INC_STEPS and h == 1:
                            ins.then_inc(dve_done, 1)
            vector.maybe_drain_then_inc((dve_done, 1))

        @block.scalar
        def _(act):
            for c in ACT_CHUNKS:
                lo, sz = CH_OFF[c] * F, CHUNK_STEPS[c] * F
                act.dma_start(ub[:, lo:lo+sz], u_ext[:, lo:lo+sz]).then_inc(sem_act, 16)
            for p, (t0, nst, wv) in enumerate(PIECES):
                lo, sz = t0 * F, nst * F
                act.wait_ge(dve_done, wv)
                nc.scalar.activation(sb[:, lo:lo+sz], wb[:, lo:lo+sz],
                                     mybir.ActivationFunctionType.Sign)
                if p >= 1:
                    ot, onst, _ = PIECES[p - 1]
                    act.dma_start(s_ext[:, ot*F:(ot+onst)*F],
                                  sb[:, ot*F:(ot+onst)*F]).then_inc(sem_out, 16)
            act.drain()
            ot, onst, _ = PIECES[-1]
            act.dma_start(s_ext[:, ot*F:(ot+onst)*F],
                          sb[:, ot*F:(ot+onst)*F]).then_inc(sem_out, 16)

    return nc


# host prescale: u[t] = fp16(x+r) * 2^(t mod K)  (exact exponent shift)
_SCALE16 = np.exp2(np.arange(T, dtype=np.float32) % K).astype(np.float16)


def _shard(inp: np.ndarray, rec: np.ndarray) -> list[dict[str, np.ndarray]]:
    u16 = (inp + rec).astype(np.float16) * _SCALE16[:, None, None]
    in_maps = []
    for i in range(NCORES):
        uc = u16[:, i*B_LOC:(i+1)*B_LOC, :].reshape(T, P, F)
        in_maps.append({"u": np.ascontiguousarray(uc.transpose(1, 0, 2)).reshape(P, T * F)})
    return in_maps


def kernel(inp: np.ndarray, rec: np.ndarray) -> np.ndarray:
    inp = np.asarray(inp, dtype=np.float32)
    rec = np.asarray(rec, dtype=np.float32)
    nc = _build_nc()
    in_maps = _shard(inp, rec)
    res = run_bass_kernel_spmd(nc, in_maps, list(range(NCORES)))
    outs = []
    for i in range(NCORES):
        raw = res.results[i]["s"].reshape(P, T, F)           # uint8
        s = (raw == 1).astype(np.float32).transpose(1, 0, 2)  # [T, P, F]
        outs.append(s.reshape(T, B_LOC, N))
    return np.concatenate(outs, axis=1)
